# revision 13
# baseline (speedup 1.0000x reference)
"""BoT tokenizer kernel for Trainium2 (Bass/Tile), 8-core data parallel.

All 25 output tokens are computed on the TensorEngine as bf16 matmuls with
an exact fp32 -> 2x bf16 mantissa split (8+8 = 16 bits):

    x = a0 + a1,  w = w0 + w1   (bf16 splits, exact by construction)
    x*w ~= a0*w0 + a0*w1 + a1*w0    (dropped a1*w1 term is ~2^-18 relative)

 - single-feature token k: K=5 matmul (3 cross products + 2 bias rows
   against a ones column)
 - fore token: 9 features -> K = 9*3 + 2 = 29
 - palm token: 7 features -> K = 7*3 + 2 = 23

The device writes the output as fp16 (26.2 MB/core instead of 52.4 MB
fp32) and the host upcasts to fp32; fp16 rounding adds ~2e-4 l2 relative
error, far inside the 2e-2 tolerance, and halves the output-DMA traffic
that bounds this kernel (~358 GB/s HBM per core -> ~73 us floor).

PSUM->SBUF conversion copies are batched 4 tokens (4 PSUM banks, one
[128, 2048] read) per instruction to amortize the fixed per-op cost, and
split across VectorE and ScalarE.
"""

import numpy as np

FORE_IDX = [0, 1, 2, 27, 28, 32, 33, 34, 38]
PALM_IDX = [4, 29, 30, 31, 35, 36, 37]
SINGLE_IDX = [3] + list(range(5, 27))

B = 8192
D = 512
T = 25
N_CORES = 8
B_LOC = B // N_CORES          # 1024 rows per core
CHUNK = 128
N_CHUNKS = B_LOC // CHUNK     # 8
ROW = T * D                   # 12800
NS = 23

KF = 9 * 3 + 2                # 29
KP = 7 * 3 + 2                # 23
KS = 5
# singles packed 3 per tile at 32-partition offsets (matmul base partition
# must be 32-aligned); tile partition extent 64+KS
S_TILES = [(a, min(a + 3, NS)) for a in range(0, NS, 3)]
S_STRIDE = 32

# PSUM copy groups: tokens [2g, 2g+2) (last group is token 24 alone);
# groups 0-5 fill DMA tile A (tokens 0-11), groups 6-12 fill tile B (12-24)
N_GROUPS = 13
DMA_A_TOK = 12                # tokens 0..11  -> [128, 6144] f16
DMA_B_TOK = 13                # tokens 12..24 -> [128, 6656] f16

_prog_cache = {}


def _k_of_tok(t):
    return 0 if t == 1 else t - 2


def _build_program():
    import concourse.bacc as bacc
    import concourse.mybir as mybir
    import concourse.tile as tile
    from concourse.bass import ts

    f32 = mybir.dt.float32
    f16 = mybir.dt.float16
    bf16 = mybir.dt.bfloat16
    nc = bacc.Bacc("TRN2", target_bir_lowering=False, debug=False,
                   num_devices=N_CORES)

    NT = len(S_TILES)             # 8 singles tiles
    S_EXT = S_STRIDE * 2 + KS     # 69 partitions per singles tile
    # fore rows at partition 0, palm rows at partition 32 (matmul base
    # partition must be 32-aligned)
    lfp_d = nc.dram_tensor("lfp", [S_STRIDE + KP, B_LOC], bf16,
                           kind="ExternalInput")
    rfp_d = nc.dram_tensor("rfp", [S_STRIDE + KP, D], bf16,
                           kind="ExternalInput")
    # all 8 singles tiles merged along the free dim: one DMA each way
    ls_d = nc.dram_tensor("lsx", [S_EXT, NT * B_LOC], bf16,
                          kind="ExternalInput")
    rs_d = nc.dram_tensor("rsx", [S_EXT, NT * D], bf16,
                          kind="ExternalInput")
    out_d = nc.dram_tensor("out", [B_LOC, ROW], f16, kind="ExternalOutput")

    with tile.TileContext(nc) as tc:
        with (
            tc.tile_pool(name="cst", bufs=1) as cst,
            tc.tile_pool(name="op", bufs=1) as op,
            tc.tile_pool(name="pp", bufs=4, space="PSUM") as pp,
        ):
            # split the loads per singles-tile (small thin-tile DMAs reach
            # few SDMA engines; one big one would gate all singles tokens)
            lfp_s = cst.tile([S_STRIDE + KP, B_LOC], bf16)
            nc.sync.dma_start(out=lfp_s[:], in_=lfp_d[:])
            rfp_s = cst.tile([S_STRIDE + KP, D], bf16)
            nc.scalar.dma_start(out=rfp_s[:], in_=rfp_d[:])
            ls_s = cst.tile([S_EXT, NT * B_LOC], bf16)
            rs_s = cst.tile([S_EXT, NT * D], bf16)
            for i in range(NT):
                le = nc.sync if i % 2 == 0 else nc.scalar
                re = nc.scalar if i % 2 == 0 else nc.sync
                le.dma_start(
                    out=ls_s[:, ts(i, B_LOC)], in_=ls_d[:, ts(i, B_LOC)])
                re.dma_start(
                    out=rs_s[:, ts(i, D)], in_=rs_d[:, ts(i, D)])

            for c in range(N_CHUNKS):
                o_a = op.tile([CHUNK, DMA_A_TOK * D], f16, tag="oA", bufs=5)
                o_b = op.tile([CHUNK, DMA_B_TOK * D], f16, tag="oB", bufs=5)
                for g in range(N_GROUPS):
                    t0, t1 = 2 * g, min(2 * g + 2, T)
                    p_t = pp.tile([CHUNK, 2 * D], f32)
                    for t in range(t0, t1):
                        if t == 0:
                            lhsT = lfp_s[0:KF, ts(c, CHUNK)]
                            rhs = rfp_s[0:KF, :]
                        elif t == 2:
                            lhsT = lfp_s[S_STRIDE:S_STRIDE + KP, ts(c, CHUNK)]
                            rhs = rfp_s[S_STRIDE:S_STRIDE + KP, :]
                        else:
                            k = _k_of_tok(t)
                            i, j = k // 3, k % 3
                            off = S_STRIDE * j
                            b0 = i * B_LOC + c * CHUNK
                            lhsT = ls_s[off:off + KS, b0:b0 + CHUNK]
                            rhs = rs_s[off:off + KS, i * D:(i + 1) * D]
                        nc.tensor.matmul(p_t[:, ts(t - t0, D)], lhsT, rhs,
                                         start=True, stop=True)
                    w = (t1 - t0) * D
                    if g < 6:
                        dst = o_a[:, 2 * g * D:2 * g * D + w]
                    else:
                        dst = o_b[:, (2 * g - DMA_A_TOK) * D:
                                  (2 * g - DMA_A_TOK) * D + w]
                    if g % 2 == 0 and g != 12:
                        nc.vector.tensor_copy(dst, p_t[:, :w])
                    else:
                        nc.scalar.copy(dst, p_t[:, :w])
                nc.sync.dma_start(
                    out=out_d[ts(c, CHUNK), 0:DMA_A_TOK * D], in_=o_a[:])
                nc.scalar.dma_start(
                    out=out_d[ts(c, CHUNK), DMA_A_TOK * D:ROW], in_=o_b[:])

    nc.compile()
    return nc


def _split2(v):
    """Exact-ish fp32 -> (bf16, bf16) mantissa split: v ~= s0+s1."""
    import ml_dtypes
    bf = ml_dtypes.bfloat16
    v = np.asarray(v, np.float32)
    s0 = v.astype(bf)
    r1 = v - s0.astype(np.float32)
    s1 = r1.astype(bf)
    return s0, s1


def _lhs_rows(xcols):
    """lhsT rows for a feature block: a0,a0,a1 per feature.

    xcols: [B, F] fp32 -> [3F, B] bf16"""
    import ml_dtypes
    Bn, F = xcols.shape
    s0, s1 = _split2(xcols)              # each [B, F]
    out = np.empty((F, 3, Bn), dtype=ml_dtypes.bfloat16)
    out[:, 0, :] = s0.T
    out[:, 1, :] = s0.T
    out[:, 2, :] = s1.T
    return out.reshape(3 * F, Bn)


def _rhs_rows(wcols):
    """rhs rows for a feature block: w0,w1,w0 per feature.

    wcols: [F, D] fp32 -> [3F, D] bf16"""
    import ml_dtypes
    F, Dn = wcols.shape
    s0, s1 = _split2(wcols)
    out = np.empty((F, 3, Dn), dtype=ml_dtypes.bfloat16)
    out[:, 0, :] = s0
    out[:, 1, :] = s1
    out[:, 2, :] = s0
    return out.reshape(3 * F, Dn)


def _host_prep(x, Wf, bf_, Wp, bp, Ws, bs):
    import ml_dtypes
    bf16 = ml_dtypes.bfloat16

    ones2 = np.ones((2, B), dtype=bf16)

    def bias_rows(bias):
        b0, b1 = _split2(bias)           # [D] each
        return np.stack([b0, b1])        # [2, D]

    # fore rows at partition 0, palm rows at partition 32
    lfp = np.zeros((S_STRIDE + KP, B), dtype=bf16)
    rfp = np.zeros((S_STRIDE + KP, D), dtype=bf16)
    lfp[0:KF] = np.concatenate([_lhs_rows(x[:, FORE_IDX]), ones2])
    rfp[0:KF] = np.concatenate([_rhs_rows(np.asarray(Wf.T)), bias_rows(bf_)])
    lfp[S_STRIDE:] = np.concatenate([_lhs_rows(x[:, PALM_IDX]), ones2])
    rfp[S_STRIDE:] = np.concatenate([_rhs_rows(np.asarray(Wp.T)),
                                     bias_rows(bp)])

    # singles: per sensor a [5, *] block at 32-partition slots, 8 tiles
    # merged along the free dim: ls[p, i, b], rs[p, i, d]
    NT = len(S_TILES)
    S_EXT = S_STRIDE * 2 + KS
    ls = np.zeros((S_EXT, NT, B), dtype=bf16)
    rs = np.zeros((S_EXT, NT, D), dtype=bf16)
    xs = x[:, SINGLE_IDX]                # [B, 23]
    for k in range(NS):
        i, j = k // 3, k % 3
        o = S_STRIDE * j
        ls[o:o + 3, i] = _lhs_rows(xs[:, k:k + 1])
        ls[o + 3:o + KS, i] = ones2
        rs[o:o + 3, i] = _rhs_rows(Ws[k:k + 1])
        rs[o + 3:o + KS, i] = bias_rows(bs[k])
    return lfp, rfp, ls, rs


def kernel(x, Wf, bf, Wp, bp, Ws, bs, _trace=False, _spmd_kwargs=None):
    from concourse.bass_utils import run_bass_kernel_spmd

    x = np.asarray(x, np.float32)
    lfp, rfp, ls, rs = _host_prep(
        x, np.asarray(Wf, np.float32), np.asarray(bf, np.float32),
        np.asarray(Wp, np.float32), np.asarray(bp, np.float32),
        np.asarray(Ws, np.float32), np.asarray(bs, np.float32))

    if "nc" not in _prog_cache:
        _prog_cache["nc"] = _build_program()
    nc = _prog_cache["nc"]

    NT = len(S_TILES)
    rsx = np.ascontiguousarray(rs.reshape(rs.shape[0], NT * D))
    in_maps = []
    for i in range(N_CORES):
        sl = slice(i * B_LOC, (i + 1) * B_LOC)
        m = {
            "lfp": np.ascontiguousarray(lfp[:, sl]),
            "rfp": rfp,
            "lsx": np.ascontiguousarray(
                ls[:, :, sl].reshape(ls.shape[0], NT * B_LOC)),
            "rsx": rsx,
        }
        in_maps.append(m)

    kwargs = dict(_spmd_kwargs or {})
    res = run_bass_kernel_spmd(nc, in_maps, core_ids=list(range(N_CORES)),
                               trace=_trace, **kwargs)
    out = np.concatenate(
        [np.asarray(r["out"]).astype(np.float32) for r in res.results],
        axis=0)
    if _trace:
        kernel.last_results = res
    return out.reshape(B, T, D)


# revision 15
# speedup vs baseline: 1.1562x; 1.1562x over previous
"""BoT tokenizer kernel for Trainium2 (Bass/Tile), 8-core data parallel.

All 25 output tokens are computed on the TensorEngine as bf16 matmuls with
an exact fp32 -> 2x bf16 mantissa split (8+8 = 16 bits):

    x = a0 + a1,  w = w0 + w1   (bf16 splits, exact by construction)
    x*w ~= a0*w0 + a0*w1 + a1*w0    (dropped a1*w1 term is ~2^-18 relative)

 - single-feature token k: K=5 matmul (3 cross products + 2 bias rows
   against a ones column)
 - fore token: 9 features -> K = 9*3 + 2 = 29
 - palm token: 7 features -> K = 7*3 + 2 = 23

The device writes the output as fp16 (26.2 MB/core instead of 52.4 MB
fp32) and the host upcasts to fp32; fp16 rounding adds ~2e-4 l2 relative
error, far inside the 2e-2 tolerance, and halves the output-DMA traffic
that bounds this kernel (~358 GB/s HBM per core -> ~73 us floor).

PSUM->SBUF conversion copies are batched 4 tokens (4 PSUM banks, one
[128, 2048] read) per instruction to amortize the fixed per-op cost, and
split across VectorE and ScalarE.
"""

import numpy as np

FORE_IDX = [0, 1, 2, 27, 28, 32, 33, 34, 38]
PALM_IDX = [4, 29, 30, 31, 35, 36, 37]
SINGLE_IDX = [3] + list(range(5, 27))

B = 8192
D = 512
T = 25
N_CORES = 8
B_LOC = B // N_CORES          # 1024 rows per core
CHUNK = 128
N_CHUNKS = B_LOC // CHUNK     # 8
ROW = T * D                   # 12800
NS = 23

KF = 9 * 3 + 2                # 29
KP = 7 * 3 + 2                # 23
KS = 5
# singles packed 3 per tile at 32-partition offsets (matmul base partition
# must be 32-aligned); tile partition extent 64+KS
S_TILES = [(a, min(a + 3, NS)) for a in range(0, NS, 3)]
S_STRIDE = 32

# PSUM copy groups: tokens [2g, 2g+2) (last group is token 24 alone);
# groups 0-5 fill DMA tile A (tokens 0-11), groups 6-12 fill tile B (12-24)
N_GROUPS = 13
DMA_A_TOK = 12                # tokens 0..11  -> [128, 6144] f16
DMA_B_TOK = 13                # tokens 12..24 -> [128, 6656] f16

_prog_cache = {}


def _k_of_tok(t):
    return 0 if t == 1 else t - 2


def _build_program():
    import concourse.bacc as bacc
    import concourse.mybir as mybir
    import concourse.tile as tile
    from concourse.bass import ts

    f32 = mybir.dt.float32
    f16 = mybir.dt.float16
    bf16 = mybir.dt.bfloat16
    nc = bacc.Bacc("TRN2", target_bir_lowering=False, debug=False,
                   num_devices=N_CORES)

    NT = len(S_TILES)             # 8 singles tiles
    S_EXT = S_STRIDE * 2 + KS     # 69 partitions per singles tile
    # fore rows at partition 0, palm rows at partition 32 (matmul base
    # partition must be 32-aligned)
    lfp_d = nc.dram_tensor("lfp", [S_STRIDE + KP, B_LOC], bf16,
                           kind="ExternalInput")
    rfp_d = nc.dram_tensor("rfp", [S_STRIDE + KP, D], bf16,
                           kind="ExternalInput")
    # all 8 singles tiles merged along the free dim: one DMA each way
    ls_d = nc.dram_tensor("lsx", [S_EXT, NT * B_LOC], bf16,
                          kind="ExternalInput")
    rs_d = nc.dram_tensor("rsx", [S_EXT, NT * D], bf16,
                          kind="ExternalInput")
    out_d = nc.dram_tensor("out", [B_LOC, ROW], f16, kind="ExternalOutput")

    with tile.TileContext(nc) as tc:
        with (
            tc.tile_pool(name="cst", bufs=1) as cst,
            tc.tile_pool(name="op", bufs=1) as op,
            tc.tile_pool(name="pp", bufs=4, space="PSUM") as pp,
        ):
            # split the loads per singles-tile (small thin-tile DMAs reach
            # few SDMA engines; one big one would gate all singles tokens)
            lfp_s = cst.tile([S_STRIDE + KP, B_LOC], bf16)
            nc.sync.dma_start(out=lfp_s[:], in_=lfp_d[:])
            rfp_s = cst.tile([S_STRIDE + KP, D], bf16)
            nc.scalar.dma_start(out=rfp_s[:], in_=rfp_d[:])
            ls_s = cst.tile([S_EXT, NT * B_LOC], bf16)
            rs_s = cst.tile([S_EXT, NT * D], bf16)
            for i in range(NT):
                le = nc.sync if i % 2 == 0 else nc.scalar
                re = nc.scalar if i % 2 == 0 else nc.sync
                le.dma_start(
                    out=ls_s[:, ts(i, B_LOC)], in_=ls_d[:, ts(i, B_LOC)])
                re.dma_start(
                    out=rs_s[:, ts(i, D)], in_=rs_d[:, ts(i, D)])

            for c in range(N_CHUNKS):
                o_t = op.tile([CHUNK, ROW], f16, tag="ot", bufs=5)
                for g in range(N_GROUPS):
                    t0, t1 = 2 * g, min(2 * g + 2, T)
                    p_t = pp.tile([CHUNK, 2 * D], f32)
                    for t in range(t0, t1):
                        if t == 0:
                            lhsT = lfp_s[0:KF, ts(c, CHUNK)]
                            rhs = rfp_s[0:KF, :]
                        elif t == 2:
                            lhsT = lfp_s[S_STRIDE:S_STRIDE + KP, ts(c, CHUNK)]
                            rhs = rfp_s[S_STRIDE:S_STRIDE + KP, :]
                        else:
                            k = _k_of_tok(t)
                            i, j = k // 3, k % 3
                            off = S_STRIDE * j
                            b0 = i * B_LOC + c * CHUNK
                            lhsT = ls_s[off:off + KS, b0:b0 + CHUNK]
                            rhs = rs_s[off:off + KS, i * D:(i + 1) * D]
                        nc.tensor.matmul(p_t[:, ts(t - t0, D)], lhsT, rhs,
                                         start=True, stop=True)
                    w = (t1 - t0) * D
                    dst = o_t[:, 2 * g * D:2 * g * D + w]
                    if g % 2 == 0 and g != 12:
                        nc.vector.tensor_copy(dst, p_t[:, :w])
                    else:
                        nc.scalar.copy(dst, p_t[:, :w])
                dma_eng = nc.sync if c % 2 == 0 else nc.scalar
                dma_eng.dma_start(out=out_d[ts(c, CHUNK), :], in_=o_t[:])

    nc.compile()
    return nc


def _split2(v):
    """Exact-ish fp32 -> (bf16, bf16) mantissa split: v ~= s0+s1."""
    import ml_dtypes
    bf = ml_dtypes.bfloat16
    v = np.asarray(v, np.float32)
    s0 = v.astype(bf)
    r1 = v - s0.astype(np.float32)
    s1 = r1.astype(bf)
    return s0, s1


def _lhs_rows(xcols):
    """lhsT rows for a feature block: a0,a0,a1 per feature.

    xcols: [B, F] fp32 -> [3F, B] bf16"""
    import ml_dtypes
    Bn, F = xcols.shape
    s0, s1 = _split2(xcols)              # each [B, F]
    out = np.empty((F, 3, Bn), dtype=ml_dtypes.bfloat16)
    out[:, 0, :] = s0.T
    out[:, 1, :] = s0.T
    out[:, 2, :] = s1.T
    return out.reshape(3 * F, Bn)


def _rhs_rows(wcols):
    """rhs rows for a feature block: w0,w1,w0 per feature.

    wcols: [F, D] fp32 -> [3F, D] bf16"""
    import ml_dtypes
    F, Dn = wcols.shape
    s0, s1 = _split2(wcols)
    out = np.empty((F, 3, Dn), dtype=ml_dtypes.bfloat16)
    out[:, 0, :] = s0
    out[:, 1, :] = s1
    out[:, 2, :] = s0
    return out.reshape(3 * F, Dn)


def _host_prep(x, Wf, bf_, Wp, bp, Ws, bs):
    import ml_dtypes
    bf16 = ml_dtypes.bfloat16

    ones2 = np.ones((2, B), dtype=bf16)

    def bias_rows(bias):
        b0, b1 = _split2(bias)           # [D] each
        return np.stack([b0, b1])        # [2, D]

    # fore rows at partition 0, palm rows at partition 32
    lfp = np.zeros((S_STRIDE + KP, B), dtype=bf16)
    rfp = np.zeros((S_STRIDE + KP, D), dtype=bf16)
    lfp[0:KF] = np.concatenate([_lhs_rows(x[:, FORE_IDX]), ones2])
    rfp[0:KF] = np.concatenate([_rhs_rows(np.asarray(Wf.T)), bias_rows(bf_)])
    lfp[S_STRIDE:] = np.concatenate([_lhs_rows(x[:, PALM_IDX]), ones2])
    rfp[S_STRIDE:] = np.concatenate([_rhs_rows(np.asarray(Wp.T)),
                                     bias_rows(bp)])

    # singles: per sensor a [5, *] block at 32-partition slots, 8 tiles
    # merged along the free dim: ls[p, i, b], rs[p, i, d]
    NT = len(S_TILES)
    S_EXT = S_STRIDE * 2 + KS
    ls = np.zeros((S_EXT, NT, B), dtype=bf16)
    rs = np.zeros((S_EXT, NT, D), dtype=bf16)
    xs = x[:, SINGLE_IDX]                # [B, 23]
    for k in range(NS):
        i, j = k // 3, k % 3
        o = S_STRIDE * j
        ls[o:o + 3, i] = _lhs_rows(xs[:, k:k + 1])
        ls[o + 3:o + KS, i] = ones2
        rs[o:o + 3, i] = _rhs_rows(Ws[k:k + 1])
        rs[o + 3:o + KS, i] = bias_rows(bs[k])
    return lfp, rfp, ls, rs


def kernel(x, Wf, bf, Wp, bp, Ws, bs, _trace=False, _spmd_kwargs=None):
    from concourse.bass_utils import run_bass_kernel_spmd

    x = np.asarray(x, np.float32)
    lfp, rfp, ls, rs = _host_prep(
        x, np.asarray(Wf, np.float32), np.asarray(bf, np.float32),
        np.asarray(Wp, np.float32), np.asarray(bp, np.float32),
        np.asarray(Ws, np.float32), np.asarray(bs, np.float32))

    if "nc" not in _prog_cache:
        _prog_cache["nc"] = _build_program()
    nc = _prog_cache["nc"]

    NT = len(S_TILES)
    rsx = np.ascontiguousarray(rs.reshape(rs.shape[0], NT * D))
    in_maps = []
    for i in range(N_CORES):
        sl = slice(i * B_LOC, (i + 1) * B_LOC)
        m = {
            "lfp": np.ascontiguousarray(lfp[:, sl]),
            "rfp": rfp,
            "lsx": np.ascontiguousarray(
                ls[:, :, sl].reshape(ls.shape[0], NT * B_LOC)),
            "rsx": rsx,
        }
        in_maps.append(m)

    kwargs = dict(_spmd_kwargs or {})
    res = run_bass_kernel_spmd(nc, in_maps, core_ids=list(range(N_CORES)),
                               trace=_trace, **kwargs)
    out = np.concatenate(
        [np.asarray(r["out"]).astype(np.float32) for r in res.results],
        axis=0)
    if _trace:
        kernel.last_results = res
    return out.reshape(B, T, D)


# revision 27
# speedup vs baseline: 1.1831x; 1.0233x over previous
"""BoT tokenizer kernel for Trainium2 (Bass/Tile), 8-core data parallel.

All 25 output tokens are computed on the TensorEngine as bf16 matmuls with
an exact fp32 -> 2x bf16 mantissa split (8+8 = 16 bits):

    x = a0 + a1,  w = w0 + w1   (bf16 splits, exact by construction)
    x*w ~= a0*w0 + a0*w1 + a1*w0    (dropped a1*w1 term is ~2^-18 relative)

 - single-feature token k: K=5 matmul (3 cross products + 2 bias rows
   against a ones column)
 - fore token: 9 features -> K = 9*3 + 2 = 29
 - palm token: 7 features -> K = 7*3 + 2 = 23

The device writes the output as fp16 (26.2 MB/core instead of 52.4 MB
fp32) and the host upcasts to fp32; fp16 rounding adds ~2e-4 l2 relative
error, far inside the 2e-2 tolerance, and halves the output-DMA traffic
that bounds this kernel (~358 GB/s HBM per core -> ~73 us floor).

PSUM->SBUF conversion copies are batched 4 tokens (4 PSUM banks, one
[128, 2048] read) per instruction to amortize the fixed per-op cost, and
split across VectorE and ScalarE.
"""

import numpy as np

FORE_IDX = [0, 1, 2, 27, 28, 32, 33, 34, 38]
PALM_IDX = [4, 29, 30, 31, 35, 36, 37]
SINGLE_IDX = [3] + list(range(5, 27))

B = 8192
D = 512
T = 25
N_CORES = 8
B_LOC = B // N_CORES          # 1024 rows per core
CHUNK = 128
N_CHUNKS = B_LOC // CHUNK     # 8
ROW = T * D                   # 12800
NS = 23

KF = 9 * 3 + 2                # 29
KP = 7 * 3 + 2                # 23
KS = 5
# singles packed 3 per tile at 32-partition offsets (matmul base partition
# must be 32-aligned); tile partition extent 64+KS
S_TILES = [(a, min(a + 3, NS)) for a in range(0, NS, 3)]
S_STRIDE = 32

# PSUM copy groups: tokens [2g, 2g+2) (last group is token 24 alone);
# groups 0-5 fill DMA tile A (tokens 0-11), groups 6-12 fill tile B (12-24)
N_GROUPS = 13
DMA_A_TOK = 12                # tokens 0..11  -> [128, 6144] f16
DMA_B_TOK = 13                # tokens 12..24 -> [128, 6656] f16

_prog_cache = {}


def _k_of_tok(t):
    return 0 if t == 1 else t - 2


def _build_program():
    import concourse.bacc as bacc
    import concourse.mybir as mybir
    import concourse.tile as tile
    from concourse.bass import ts

    f32 = mybir.dt.float32
    f16 = mybir.dt.float16
    bf16 = mybir.dt.bfloat16
    nc = bacc.Bacc("TRN2", target_bir_lowering=False, debug=False,
                   num_devices=N_CORES)

    NT = len(S_TILES)             # 8 singles tiles
    S_EXT = S_STRIDE * 2 + KS     # 69 partitions per singles tile
    # fore rows at partition 0, palm rows at partition 32 (matmul base
    # partition must be 32-aligned)
    lfp_d = nc.dram_tensor("lfp", [S_STRIDE + KP, B_LOC], bf16,
                           kind="ExternalInput")
    rfp_d = nc.dram_tensor("rfp", [S_STRIDE + KP, D], bf16,
                           kind="ExternalInput")
    # singles inputs arrive slot-major: row 5*j+r holds slot j's row r for
    # all 8 tiles along the free dim, so each slot loads as one plain DMA
    ls_d = nc.dram_tensor("lsd", [3 * KS, NT * B_LOC], bf16,
                          kind="ExternalInput")
    rs_d = nc.dram_tensor("rsd", [3 * KS, NT * D], bf16,
                          kind="ExternalInput")
    out_d = nc.dram_tensor("out", [B_LOC, ROW], f16, kind="ExternalOutput")

    with tile.TileContext(nc) as tc:
        with (
            tc.tile_pool(name="cst", bufs=1) as cst,
            tc.tile_pool(name="op", bufs=1) as op,
            tc.tile_pool(name="pp", bufs=4, space="PSUM") as pp,
        ):
            lfp_s = cst.tile([S_STRIDE + KP, B_LOC], bf16)
            nc.sync.dma_start(out=lfp_s[:], in_=lfp_d[:])
            rfp_s = cst.tile([S_STRIDE + KP, D], bf16)
            nc.scalar.dma_start(out=rfp_s[:], in_=rfp_d[:])
            # scatter the slot-major singles rows into 32-aligned matmul
            # slots: one plain 2D DMA per slot (DMA is address-based, so it
            # can place rows at shifted partitions; engine copies cannot)
            ls_s = cst.tile([S_EXT, NT * B_LOC], bf16)
            rs_s = cst.tile([S_EXT, NT * D], bf16)
            for j in range(3):
                nc.sync.dma_start(
                    out=ls_s[S_STRIDE * j:S_STRIDE * j + KS, :],
                    in_=ls_d[KS * j:KS * (j + 1), :])
                nc.scalar.dma_start(
                    out=rs_s[S_STRIDE * j:S_STRIDE * j + KS, :],
                    in_=rs_d[KS * j:KS * (j + 1), :])

            for c in range(N_CHUNKS):
                o_t = op.tile([CHUNK, ROW], f16, tag="ot", bufs=5)
                for g in range(N_GROUPS):
                    t0, t1 = 2 * g, min(2 * g + 2, T)
                    p_t = pp.tile([CHUNK, 2 * D], f32)
                    for t in range(t0, t1):
                        if t == 0:
                            lhsT = lfp_s[0:KF, ts(c, CHUNK)]
                            rhs = rfp_s[0:KF, :]
                        elif t == 2:
                            lhsT = lfp_s[S_STRIDE:S_STRIDE + KP, ts(c, CHUNK)]
                            rhs = rfp_s[S_STRIDE:S_STRIDE + KP, :]
                        else:
                            k = _k_of_tok(t)
                            i, j = k // 3, k % 3
                            off = S_STRIDE * j
                            b0 = i * B_LOC + c * CHUNK
                            lhsT = ls_s[off:off + KS, b0:b0 + CHUNK]
                            rhs = rs_s[off:off + KS, i * D:(i + 1) * D]
                        nc.tensor.matmul(p_t[:, ts(t - t0, D)], lhsT, rhs,
                                         start=True, stop=True)
                    w = (t1 - t0) * D
                    dst = o_t[:, 2 * g * D:2 * g * D + w]
                    if g % 2 == 0 and g != 12:
                        nc.vector.tensor_copy(dst, p_t[:, :w])
                    else:
                        nc.scalar.copy(dst, p_t[:, :w])
                dma_eng = nc.sync if c % 2 == 0 else nc.scalar
                if c == 0 or c == N_CHUNKS - 1:
                    # first chunk: start draining before all copies finish;
                    # last chunk: overlap the drain with the tail copies
                    for w0, w1 in ((0, 10), (10, 18), (18, T)):
                        dma_eng.dma_start(
                            out=out_d[ts(c, CHUNK), w0 * D:w1 * D],
                            in_=o_t[:, w0 * D:w1 * D])
                else:
                    dma_eng.dma_start(out=out_d[ts(c, CHUNK), :], in_=o_t[:])

    nc.compile()
    return nc


def _split2(v):
    """Exact-ish fp32 -> (bf16, bf16) mantissa split: v ~= s0+s1."""
    import ml_dtypes
    bf = ml_dtypes.bfloat16
    v = np.asarray(v, np.float32)
    s0 = v.astype(bf)
    r1 = v - s0.astype(np.float32)
    s1 = r1.astype(bf)
    return s0, s1


def _lhs_rows(xcols):
    """lhsT rows for a feature block: a0,a0,a1 per feature.

    xcols: [B, F] fp32 -> [3F, B] bf16"""
    import ml_dtypes
    Bn, F = xcols.shape
    s0, s1 = _split2(xcols)              # each [B, F]
    out = np.empty((F, 3, Bn), dtype=ml_dtypes.bfloat16)
    out[:, 0, :] = s0.T
    out[:, 1, :] = s0.T
    out[:, 2, :] = s1.T
    return out.reshape(3 * F, Bn)


def _rhs_rows(wcols):
    """rhs rows for a feature block: w0,w1,w0 per feature.

    wcols: [F, D] fp32 -> [3F, D] bf16"""
    import ml_dtypes
    F, Dn = wcols.shape
    s0, s1 = _split2(wcols)
    out = np.empty((F, 3, Dn), dtype=ml_dtypes.bfloat16)
    out[:, 0, :] = s0
    out[:, 1, :] = s1
    out[:, 2, :] = s0
    return out.reshape(3 * F, Dn)


def _host_prep(x, Wf, bf_, Wp, bp, Ws, bs):
    import ml_dtypes
    bf16 = ml_dtypes.bfloat16

    ones2 = np.ones((2, B), dtype=bf16)

    def bias_rows(bias):
        b0, b1 = _split2(bias)           # [D] each
        return np.stack([b0, b1])        # [2, D]

    # fore rows at partition 0, palm rows at partition 32
    lfp = np.zeros((S_STRIDE + KP, B), dtype=bf16)
    rfp = np.zeros((S_STRIDE + KP, D), dtype=bf16)
    lfp[0:KF] = np.concatenate([_lhs_rows(x[:, FORE_IDX]), ones2])
    rfp[0:KF] = np.concatenate([_rhs_rows(np.asarray(Wf.T)), bias_rows(bf_)])
    lfp[S_STRIDE:] = np.concatenate([_lhs_rows(x[:, PALM_IDX]), ones2])
    rfp[S_STRIDE:] = np.concatenate([_rhs_rows(np.asarray(Wp.T)),
                                     bias_rows(bp)])

    # singles slot-major: ls[5j+r, i, :] = slot j row r of tile i
    NT = len(S_TILES)
    ls = np.zeros((3 * KS, NT, B), dtype=bf16)
    rs = np.zeros((3 * KS, NT, D), dtype=bf16)
    xs = x[:, SINGLE_IDX]                # [B, 23]
    for k in range(NS):
        i, j = k // 3, k % 3
        o = KS * j
        ls[o:o + 3, i] = _lhs_rows(xs[:, k:k + 1])
        ls[o + 3:o + KS, i] = ones2
        rs[o:o + 3, i] = _rhs_rows(Ws[k:k + 1])
        rs[o + 3:o + KS, i] = bias_rows(bs[k])
    return lfp, rfp, ls, rs


def kernel(x, Wf, bf, Wp, bp, Ws, bs, _trace=False, _spmd_kwargs=None):
    from concourse.bass_utils import run_bass_kernel_spmd

    x = np.asarray(x, np.float32)
    lfp, rfp, ls, rs = _host_prep(
        x, np.asarray(Wf, np.float32), np.asarray(bf, np.float32),
        np.asarray(Wp, np.float32), np.asarray(bp, np.float32),
        np.asarray(Ws, np.float32), np.asarray(bs, np.float32))

    if "nc" not in _prog_cache:
        _prog_cache["nc"] = _build_program()
    nc = _prog_cache["nc"]

    rsd = np.ascontiguousarray(rs.reshape(rs.shape[0], -1))
    in_maps = []
    for i in range(N_CORES):
        sl = slice(i * B_LOC, (i + 1) * B_LOC)
        m = {
            "lfp": np.ascontiguousarray(lfp[:, sl]),
            "rfp": rfp,
            "lsd": np.ascontiguousarray(
                ls[:, :, sl].reshape(ls.shape[0], -1)),
            "rsd": rsd,
        }
        in_maps.append(m)

    kwargs = dict(_spmd_kwargs or {})
    res = run_bass_kernel_spmd(nc, in_maps, core_ids=list(range(N_CORES)),
                               trace=_trace, **kwargs)
    out = np.concatenate(
        [np.asarray(r["out"]).astype(np.float32) for r in res.results],
        axis=0)
    if _trace:
        kernel.last_results = res
    return out.reshape(B, T, D)
